# revision 11
# baseline (speedup 1.0000x reference)
"""Trainium2 Bass kernel for nn_AttentionModel (B=4, S=4096, E=2048) on 8 cores.

Sharding: data-parallel over batch B (4) x tensor-parallel over the E output
dim of the Q projection (2). Core c handles batch b=c//2 and scores rows
e in [h*1024, (h+1)*1024) with h=c%2. Each core computes k, v in full for its
batch (duplicated within the pair; avoids collectives), q for its half, then
scores -> softmax -> attn @ v for its half of the output rows.

All GEMMs run on the PE array in float32r (full-rate fp32, ~1e-4 rel err).
Every matmul contracts over the partition dim:
  qT,kT [s, e]: stationary = transposed-x column tiles (host provides x^T)
  v     [f, s]: stationary = Wv^T column tiles, moving = x^T row-blocks
  scores[e, f] = qT.T @ kT contracting s; softmax over free dim f
  out   [e, s] = attnT.T @ v contracting f
Q/K biases enter via rank-1 (K=1) matmul accumulation; V bias via the
per-partition bias of the activation eviction; the softmax 1/rowsum is
applied as the per-partition ACT scale of the P4 eviction. The 1/sqrt(E)
score scale is folded into Wq/bq on the host.

All phases draw their large resident operand from one shared double-buffered
64KB-per-partition pool ("big"), so each phase's resident streams in while
the previous phase is still computing; per-iteration operands use a shared
16KB stream pool on the second HWDGE ring.
"""

import sys

sys.path.insert(0, "/opt/trn_rl_repo")

from contextlib import ExitStack

import numpy as np

import concourse.bass as bass
import concourse.mybir as mybir
import concourse.tile as tile
from concourse import bacc
from concourse.bass_utils import run_bass_kernel_spmd
from concourse.masks import make_identity

f32 = mybir.dt.float32
f32r = mybir.dt.float32r

B, S, E = 4, 4096, 2048
EH = E // 2          # per-core q rows (embed half)
N = 512              # moving free-dim per matmul (one PSUM bank)
SKT = S // 128       # 32 s k-tiles
EKT = E // 128       # 16 e k-tiles
N_CORES = 8


def build_kernel():
    nc = bacc.Bacc("TRN2", debug=False, target_bir_lowering=False)

    xt = nc.dram_tensor("xt", [E, S], f32r, kind="ExternalInput")         # x^T
    xtt = nc.dram_tensor("xtt", [SKT, 128, EKT, 128], f32r, kind="ExternalInput")
    wqk = nc.dram_tensor("wqk", [E, E + EH], f32r, kind="ExternalInput")  # [Wk^T | Wq_h^T/sqrt(E)]
    bkq = nc.dram_tensor("bkq", [1, E + EH], f32r, kind="ExternalInput")  # [bk | bq_h/sqrt(E)]
    wv = nc.dram_tensor("wv", [EKT, E, 128], f32r, kind="ExternalInput")  # Wv^T tiled by f
    bv = nc.dram_tensor("bv", [128, EKT], f32, kind="ExternalInput")      # bv packed per f-tile
    ones_d = nc.dram_tensor("ones", [1, 128], f32r, kind="ExternalInput")
    outt = nc.dram_tensor("outt", [EH, S], f32, kind="ExternalOutput")

    with tile.TileContext(nc) as tc, ExitStack() as ctx:
        dram = ctx.enter_context(tc.tile_pool(name="dram", bufs=1, space="DRAM"))
        qt_d = dram.tile([EH // 128, 128, SKT, 128], f32r)   # [et, s_in, skt, e_in]
        kt_d = dram.tile([S, E], f32r)
        v_d = dram.tile([E, S], f32r)
        sc_d = dram.tile([EH, E], f32)
        att_d = dram.tile([EH // 128, 128, EKT, 128], f32r)  # [et, f_in, fkt, e_in]

        const = ctx.enter_context(tc.tile_pool(name="const", bufs=1))
        ones_sb = const.tile([1, 128], f32r)
        nc.sync.dma_start(ones_sb[:, :], ones_d[:, :])
        ident = const.tile([128, 128], f32)
        make_identity(nc, ident[:, :])
        bv_sb = const.tile([128, EKT], f32)
        nc.sync.dma_start(bv_sb[:, :], bv[:, :])
        rsumall = const.tile([128, EH // 128], f32)

        pbig = ctx.enter_context(tc.tile_pool(name="pbig", bufs=2))
        pstream = ctx.enter_context(tc.tile_pool(name="pstream", bufs=2))
        pstage = ctx.enter_context(tc.tile_pool(name="pstage", bufs=3))
        pbias = ctx.enter_context(tc.tile_pool(name="pbias", bufs=2))
        psm = ctx.enter_context(tc.tile_pool(name="psm", bufs=2))
        psma = ctx.enter_context(tc.tile_pool(name="psma", bufs=1))
        pps = ctx.enter_context(tc.tile_pool(name="pps", bufs=3, space="PSUM"))
        ppst = ctx.enter_context(tc.tile_pool(name="ppst", bufs=1, space="PSUM"))

        # ---- Phase 1ab: kT [s, f] and qT [s, e_h], three 1024-wide w passes ----
        for p1p, (a, b_) in enumerate([(0, 1024), (1024, 2048), (2048, 3072)]):
            w_sb = pbig.tile([128, EKT, 1024], f32r, tag="big", name=f"w{p1p}")
            for ekt in range(EKT):
                nc.sync.dma_start(
                    w_sb[:, ekt, :], wqk[ekt * 128:(ekt + 1) * 128, a:b_]
                )
            bias_sb = pbias.tile([1, 1024], f32r, tag="bias", name=f"bias{p1p}")
            nc.sync.dma_start(bias_sb[:, :], bkq[:, a:b_])
            for st in range(SKT):
                xtc = pstream.tile([128, EKT, 128], f32r, tag="st", name=f"xtc{p1p}_{st}")
                nc.scalar.dma_start(xtc[:, :, :], xtt[st])
                ps = pps.tile([128, 1024], f32, tag="ps", name=f"ps1_{p1p}_{st}")
                for ekt in range(EKT):
                    for fc in range(2):
                        nc.tensor.matmul(
                            ps[:, fc * N:(fc + 1) * N],
                            xtc[:, ekt, :],
                            w_sb[:, ekt, fc * N:(fc + 1) * N],
                            start=(ekt == 0),
                            stop=False,
                        )
                for fc in range(2):
                    nc.tensor.matmul(
                        ps[:, fc * N:(fc + 1) * N],
                        ones_sb[:, :],
                        bias_sb[:, fc * N:(fc + 1) * N],
                        start=False,
                        stop=True,
                    )
                osb = pstage.tile([128, 1024], f32r, tag="sg", name=f"o1_{p1p}_{st}")
                nc.scalar.copy(osb[:, :], ps[:, :])
                rows = slice(st * 128, (st + 1) * 128)
                if p1p < 2:
                    nc.sync.dma_start(kt_d[rows, a:b_], osb[:, :])
                else:
                    nc.sync.dma_start(
                        qt_d[:, :, st, :].rearrange("et p e -> p et e"),
                        osb[:, :].rearrange("p (et e) -> p et e", e=128),
                    )

        # ---- Phase 1c: v [f, s] in two s-half passes ----
        for sh in range(2):
            xha = pbig.tile([128, EKT, 1024], f32r, tag="big", name=f"xh{sh}a")
            xhb = pbig.tile([128, EKT, 1024], f32r, tag="big", name=f"xh{sh}b")
            for ekt in range(EKT):
                nc.sync.dma_start(
                    xha[:, ekt, :],
                    xt[ekt * 128:(ekt + 1) * 128, sh * 2048:sh * 2048 + 1024],
                )
                nc.sync.dma_start(
                    xhb[:, ekt, :],
                    xt[ekt * 128:(ekt + 1) * 128, sh * 2048 + 1024:(sh + 1) * 2048],
                )
            for ft in range(EKT):
                wvc = pstream.tile([128, EKT, 128], f32r, tag="st", name=f"wvc{sh}_{ft}")
                nc.scalar.dma_start(
                    wvc[:, :, :], wv[ft].rearrange("(kt p) f -> p kt f", p=128)
                )
                for half, xh in ((0, xha), (1, xhb)):
                    ps = pps.tile([128, 1024], f32, tag="ps", name=f"psv{sh}_{ft}_{half}")
                    for ekt in range(EKT):
                        for sc in range(2):
                            nc.tensor.matmul(
                                ps[:, sc * N:(sc + 1) * N],
                                wvc[:, ekt, :],
                                xh[:, ekt, sc * N:(sc + 1) * N],
                                start=(ekt == 0),
                                stop=(ekt == EKT - 1),
                            )
                    vsb = pstage.tile([128, 1024], f32r, tag="sg", name=f"vsb{sh}_{ft}_{half}")
                    nc.scalar.activation(
                        vsb[:, :], ps[:, :],
                        mybir.ActivationFunctionType.Identity,
                        bias=bv_sb[:, ft:ft + 1], scale=1.0,
                    )
                    nc.sync.dma_start(
                        v_d[ft * 128:(ft + 1) * 128,
                            sh * 2048 + half * 1024:sh * 2048 + (half + 1) * 1024],
                        vsb[:, :],
                    )

        # ---- Phase 2: scores [e_h, f] in two f-half passes ----
        for fh in range(2):
            ka = pbig.tile([128, SKT, 512], f32r, tag="big", name=f"k{fh}a")
            kb = pbig.tile([128, SKT, 512], f32r, tag="big", name=f"k{fh}b")
            for skt in range(SKT):
                r = slice(skt * 128, (skt + 1) * 128)
                nc.sync.dma_start(ka[:, skt, :], kt_d[r, fh * 1024:fh * 1024 + 512])
                nc.sync.dma_start(kb[:, skt, :], kt_d[r, fh * 1024 + 512:(fh + 1) * 1024])
            for et in range(EH // 128):
                qtc = pstream.tile([128, SKT, 128], f32r, tag="st", name=f"qtc{fh}_{et}")
                nc.scalar.dma_start(qtc[:, :, :], qt_d[et])
                ps = pps.tile([128, 1024], f32, tag="ps", name=f"ps2_{fh}_{et}")
                for skt in range(SKT):
                    nc.tensor.matmul(
                        ps[:, 0:N], qtc[:, skt, :], ka[:, skt, :],
                        start=(skt == 0), stop=(skt == SKT - 1),
                    )
                    nc.tensor.matmul(
                        ps[:, N:2 * N], qtc[:, skt, :], kb[:, skt, :],
                        start=(skt == 0), stop=(skt == SKT - 1),
                    )
                ssb = pstage.tile([128, 1024], f32, tag="sg", name=f"ssb{fh}_{et}")
                nc.scalar.copy(ssb[:, :], ps[:, :])
                nc.sync.dma_start(
                    sc_d[et * 128:(et + 1) * 128, fh * 1024:(fh + 1) * 1024],
                    ssb[:, :],
                )

        # ---- Phase 3: softmax rows + attn^T tiles to DRAM ----
        for et in range(EH // 128):
            scs = psm.tile([128, E], f32, tag="scs", name=f"scs{et}")
            nc.scalar.dma_start(scs[:, :], sc_d[et * 128:(et + 1) * 128, :])
            negmax = psm.tile([128, 1], f32, tag="negmax", name=f"nm{et}")
            nc.vector.tensor_reduce(
                out=negmax[:, :], in_=scs[:, :], op=mybir.AluOpType.max,
                axis=mybir.AxisListType.X, negate=True,
            )
            attn = psma.tile([128, E], f32, tag="attn", name=f"attn{et}")
            sums = psm.tile([128, 1], f32, tag="sums", name=f"sums{et}")
            nc.scalar.activation(
                attn[:, :], scs[:, :], mybir.ActivationFunctionType.Exp,
                bias=negmax[:, 0:1], scale=1.0, accum_out=sums[:, 0:1],
            )
            nc.vector.reciprocal(rsumall[:, et:et + 1], sums[:, :])
            for half in range(2):
                pst = ppst.tile([128, 1024], f32, tag="pst", name=f"pst{et}_{half}")
                for c in range(8):
                    fkt = half * 8 + c
                    nc.tensor.transpose(
                        pst[:, c * 128:(c + 1) * 128],
                        attn[:, fkt * 128:(fkt + 1) * 128],
                        ident[:, :],
                    )
                stg = pstage.tile([128, 1024], f32r, tag="sg", name=f"stg{et}_{half}")
                nc.vector.tensor_copy(stg[:, :], pst[:, :])
                nc.sync.dma_start(
                    att_d[et, :, half * 8:(half + 1) * 8, :],
                    stg[:, :].rearrange("p (j e) -> p j e", e=128),
                )

        # ---- Phase 4: out [e_h, s] = attnT.T @ v, four s-block passes ----
        for sb in range(4):
            vb = pbig.tile([128, EKT, 1024], f32r, tag="big", name=f"vb{sb}")
            for fkt in range(EKT):
                nc.sync.dma_start(
                    vb[:, fkt, :],
                    v_d[fkt * 128:(fkt + 1) * 128, sb * 1024:(sb + 1) * 1024],
                )
            for et in range(EH // 128):
                atc = pstream.tile([128, EKT, 128], f32r, tag="st", name=f"atc{sb}_{et}")
                nc.scalar.dma_start(atc[:, :, :], att_d[et])
                ps = pps.tile([128, 1024], f32, tag="ps", name=f"ps4_{sb}_{et}")
                for fkt in range(EKT):
                    for sc in range(2):
                        nc.tensor.matmul(
                            ps[:, sc * N:(sc + 1) * N],
                            atc[:, fkt, :],
                            vb[:, fkt, sc * N:(sc + 1) * N],
                            start=(fkt == 0),
                            stop=(fkt == EKT - 1),
                        )
                osb = pstage.tile([128, 1024], f32, tag="sg", name=f"osb{sb}_{et}")
                nc.scalar.activation(
                    osb[:, :], ps[:, :],
                    mybir.ActivationFunctionType.Identity,
                    bias=0.0, scale=rsumall[:, et:et + 1],
                )
                nc.sync.dma_start(
                    outt[et * 128:(et + 1) * 128, sb * 1024:(sb + 1) * 1024],
                    osb[:, :],
                )

    nc.compile()
    return nc


_NC_CACHE = {}


def _get_nc():
    if "nc" not in _NC_CACHE:
        _NC_CACHE["nc"] = build_kernel()
    return _NC_CACHE["nc"]


def make_in_maps(x, Wq, bq, Wk, bk, Wv, bv):
    sc = np.float32(1.0 / np.sqrt(E))
    in_maps = []
    wk_t = np.ascontiguousarray(Wk.T)                       # [E, E]
    wv_t = np.ascontiguousarray(Wv.T)                       # [E, E]
    wv_tiled = np.ascontiguousarray(
        wv_t.reshape(E, EKT, 128).transpose(1, 0, 2)        # [EKT, E, 128]
    )
    bv_packed = np.ascontiguousarray(bv.reshape(EKT, 128).T)  # [128, EKT]
    for c in range(N_CORES):
        b, h = c // 2, c % 2
        xt = np.ascontiguousarray(x[b].T)                   # [E, S]
        xtt = np.ascontiguousarray(
            x[b].reshape(SKT, 128, EKT, 128).transpose(0, 3, 2, 1)
        )                                                   # [st, e, kt, s]
        wq_h = Wq[h * EH:(h + 1) * EH, :] * sc              # [EH, E]
        wqk = np.ascontiguousarray(
            np.concatenate([wk_t, wq_h.T], axis=1)          # [E, E+EH]
        )
        bkq = np.concatenate([bk, bq[h * EH:(h + 1) * EH] * sc])[None, :]
        in_maps.append({
            "xt": xt,
            "xtt": xtt,
            "wqk": wqk,
            "bkq": np.ascontiguousarray(bkq.astype(np.float32)),
            "wv": wv_tiled,
            "bv": bv_packed,
            "ones": np.ones((1, 128), np.float32),
        })
    return in_maps


def run(in_maps, trace=False, **kwargs):
    nc = _get_nc()
    return run_bass_kernel_spmd(
        nc, in_maps, core_ids=list(range(N_CORES)), trace=trace, **kwargs
    )


def kernel(x, Wq, bq, Wk, bk, Wv, bv):
    x = np.asarray(x, dtype=np.float32)
    in_maps = make_in_maps(
        x,
        np.asarray(Wq, np.float32), np.asarray(bq, np.float32),
        np.asarray(Wk, np.float32), np.asarray(bk, np.float32),
        np.asarray(Wv, np.float32), np.asarray(bv, np.float32),
    )
    res = run(in_maps, trace=False)
    out = np.empty((B, E, S), dtype=np.float32)
    for c in range(N_CORES):
        b, h = c // 2, c % 2
        out[b, h * EH:(h + 1) * EH, :] = res.results[c]["outt"]
    return out
